# revision 4
# baseline (speedup 1.0000x reference)
"""Per-pixel dynamic 7x7 filtering (BaseTextureDiffusion._diffusion_step)
on 8 Trainium2 NeuronCores.

out[b,c,h,w] = sum_k weights[b,c,k,h,w] * pad_edge(latent)[b,c,h+i,w+j],
k = i*7+j.

Sharding: the 48 (b,c) planes are independent -> 6 planes per core.
Latent is replicate-padded on host (tiny).

Architecture (v2, all five engines balanced ~80-90us/core):
  * Weights ship as uint8 (q = round(255*w)) -> half the HBM traffic of the
    f16 baseline (19.3 MB/core vs 38.5).  The 1/255 dequant scale is folded
    into the PE identity matrix, so dequant is a plain u8->f16 copy.
  * Dequant u8->f16 runs on the otherwise-idle ScalarE (activation Copy) and
    gpsimd (tensor_copy), split ~9:5 to balance their 1.2:0.72 elem rates.
  * DVE does ONLY the 49 per-tap multiplies (f16, 2x mode) -> ~84us, vs the
    baseline's 97 passes (muls+adds) at ~162us.
  * The 48 adds run on the otherwise-idle PE: per tap, identity-matmuls
    accumulate the product tile into PSUM (start=k0/stop=k48, 3 banks of
    512 f32 per 128-row block).  Accumulation is exact f32.
  * Plane-PAIR interleaved free layout [pair(3), col, slot(2)]: a column
    shift j becomes a 4j-byte offset -> always 4B-aligned, so DVE 2x mode
    works for every tap with NO shifted latent copies (the baseline burned
    5.6 MB/core of DMA on 1-col-shifted rso tiles).
  * PSUM is evicted f32->f16 on DVE (6 small copies) and the f16 result
    DMA'd out; host upcasts to f32 (adds ~5e-4 relerr, budget is 2e-2).

Per-core engine model (cost-model ns): DMA 86us (25.7MB @ 299GB/s),
DVE 88us, ScalarE ~84us, gpsimd ~80us, PE ~65us.
"""

import numpy as np

B, C, H, W = 2, 24, 256, 256
R = 7
PAD = R // 2
NCORES = 8
PLANES = B * C  # 48
PPC = PLANES // NCORES  # 6 planes per core
NPAIR = PPC // 2  # 3 plane pairs (innermost interleave of 2)
HP = H + 2 * PAD  # 262
WP = W + 2 * PAD  # 262
FREE = NPAIR * W * 2  # 1536 elems per partition per tap
BANK = 512  # f32 elems per PSUM bank
NBANK = FREE // BANK  # 3
STAGGERED = False  # staggered sem reset on the For_i timing loop

# Dequant unit = one (block, i-row) group of 7 taps.  Units are numbered
# u = blk*7 + i in program order; these go to gpsimd, the rest to ScalarE
# (ScalarE is ~1.67x faster per element; 9:5 split balances them).
GPS_UNITS = frozenset({2, 5, 8, 11, 13})

_cache = {}


def _split_multi_waits(nc, max_waits: int = 1):
    """walrus CoreV3 codegen in this container rejects instructions carrying
    more than one sync wait ('Too many sync wait commands').  Legalize the
    module by hoisting extra waits onto same-engine NoOps inserted directly
    before the instruction (engine stalls at the nop first — semantics
    preserved, the instruction still executes only after all conditions)."""
    import concourse.mybir as mybir

    cnt = 0
    for f in nc.m.functions:
        for b in f.blocks:
            changed = False
            new_insts = []
            for inst in b.instructions:
                si = inst.sync_info
                if si is not None and len(si.on_wait) > max_waits:
                    waits = list(si.on_wait)
                    upds = list(si.on_update)
                    chunks = [
                        waits[i : i + max_waits]
                        for i in range(0, len(waits), max_waits)
                    ]
                    for chunk in chunks[:-1]:
                        nop = mybir.InstNoOp(
                            name=f"ws_nop_{cnt}", ins=[], outs=[]
                        )
                        cnt += 1
                        nop.engine = inst.engine
                        nop.sync_info = mybir.SyncInfo(
                            on_wait=chunk, on_update=[]
                        )
                        new_insts.append(nop)
                    inst.sync_info = mybir.SyncInfo(
                        on_wait=chunks[-1], on_update=upds
                    )
                    changed = True
                new_insts.append(inst)
            if changed:
                b.instructions = new_insts


def build_nc(
    reps: int = 1,
    loop_reps: int | None = None,
    skip_compute: bool = False,
):
    """Build the per-core Bass program (SPMD; all cores run the same NEFF).

    loop_reps: if set, wrap ONE rep body in a hardware For_i loop with this
    trip count (constant NEFF size for any count; used for timing).
    """
    import concourse.bass as bass
    import concourse.mybir as mybir
    from concourse.tile import TileContext

    f16 = mybir.dt.float16
    f32 = mybir.dt.float32
    u8 = mybir.dt.uint8

    nc = bass.Bass("TRN2", target_bir_lowering=False, debug=False, num_devices=NCORES)
    # Host pre-layouts (see _prep_inputs):
    #   wq: [row, tap, pair, col, slot] u8 — per (block, tap) DMA is 1536
    #       contiguous bytes per partition.
    #   lp: padded latent [prow, pair, pcol, slot] f16.
    #   out: [row, pair, col, slot] f16 (host upcasts to f32).
    wq = nc.dram_tensor("wq", [H, R * R, NPAIR, W, 2], u8, kind="ExternalInput").ap()
    lp = nc.dram_tensor("lp", [HP, NPAIR, WP, 2], f16, kind="ExternalInput").ap()
    out = nc.dram_tensor("out", [H, NPAIR, W, 2], f16, kind="ExternalOutput").ap()

    with TileContext(nc) as tc:
        with tc.tile_pool(name="pool", bufs=1) as pool, \
             tc.tile_pool(name="psum", bufs=1, space="PSUM") as psum_pool:

            def rep_body(rep):
                # identity * (1/255): the dequant scale rides the PE pass.
                ident = pool.tile([128, 128], f16, name=f"ident_{rep}",
                                  tag="ident", bufs=1)
                nc.gpsimd.memset(ident[:], 0.0)
                nc.gpsimd.affine_select(
                    out=ident[:], in_=ident[:],
                    compare_op=mybir.AluOpType.not_equal,
                    fill=1.0 / 255.0, base=0,
                    pattern=[[-1, 128]], channel_multiplier=1,
                )

                for blk in range(H // 128):
                    r0 = blk * 128
                    psum = [
                        psum_pool.tile([128, BANK], f32,
                                       name=f"ps_{rep}_{blk}_{b}",
                                       tag=f"ps{b}", bufs=2)
                        for b in range(NBANK)
                    ]
                    for i in range(R):  # i-row group: taps k = 7i + j
                        u = blk * R + i
                        # row-shifted latent tile for this group
                        rs = pool.tile([128, NPAIR, WP, 2], f16,
                                       name=f"rs_{rep}_{blk}_{i}",
                                       tag=f"rs{i}", bufs=2)
                        nc.sync.dma_start(out=rs[:], in_=lp[r0 + i : r0 + i + 128])
                        # weights: 7 per-tap DMAs into one u8 staging tile
                        wqg = pool.tile([128, R, NPAIR, W, 2], u8,
                                        name=f"wqg_{rep}_{u}", tag="wqg", bufs=3)
                        for j in range(R):
                            nc.sync.dma_start(
                                out=wqg[:, j],
                                in_=wq[r0 : r0 + 128, i * R + j],
                            )
                        if skip_compute:
                            continue
                        # dequant u8 -> f16 (plain copy; scale is in ident)
                        wf = pool.tile([128, R, NPAIR, W, 2], f16,
                                       name=f"wf_{rep}_{u}", tag="wf", bufs=2)
                        if u in GPS_UNITS:
                            nc.gpsimd.tensor_copy(wf[:], wqg[:])
                        else:
                            nc.scalar.activation(
                                wf[:], wqg[:],
                                mybir.ActivationFunctionType.Copy,
                                bias=0.0, scale=1.0,
                            )
                        for j in range(R):
                            k = i * R + j
                            prod = pool.tile([128, NPAIR, W, 2], f16,
                                             name=f"prod_{rep}_{blk}_{k}",
                                             tag="prod", bufs=4)
                            nc.vector.tensor_mul(
                                prod[:], wf[:, j], rs[:, :, j : j + W, :]
                            )
                            # bank b == plane-pair b: [128, 3, 512] view
                            pf = prod[:].rearrange("p a c s -> p a (c s)")
                            for b in range(NBANK):
                                nc.tensor.matmul(
                                    psum[b][:], lhsT=ident[:], rhs=pf[:, b],
                                    start=(k == 0), stop=(k == R * R - 1),
                                )
                    # evict PSUM (f32) to SBUF f16, then one DMA out
                    ev = pool.tile([128, NBANK, BANK], f16,
                                   name=f"ev_{rep}_{blk}", tag="ev", bufs=2)
                    if skip_compute:
                        nc.vector.memset(ev[:], 0.0)
                    else:
                        for b in range(NBANK):
                            nc.vector.tensor_copy(ev[:, b], psum[b][:])
                    nc.sync.dma_start(
                        out=out[r0 : r0 + 128],
                        in_=ev[:].rearrange("p nb (c s) -> p nb c s", c=W, s=2),
                    )

            if loop_reps is not None:
                with tc.For_i(0, loop_reps, 1, staggered_reset=STAGGERED):
                    rep_body(0)
            else:
                for rep in range(reps):
                    rep_body(rep)
    _split_multi_waits(nc)
    return nc


def _prep_inputs(latent, weights):
    """Per-core input maps.  Quantize weights to u8 (q = round(255 w)) and
    interleave plane PAIRS innermost: free layout [pair, col, slot] makes a
    column shift j a 4j-byte offset (always 4B aligned -> DVE 2x mode)."""
    lat = np.asarray(latent, dtype=np.float32).reshape(PLANES, H, W)
    wts = np.asarray(weights, dtype=np.float32).reshape(PLANES, R * R, H, W)
    lpad = np.pad(lat, ((0, 0), (PAD, PAD), (PAD, PAD)), mode="edge").astype(
        np.float16
    )
    wq_all = np.rint(wts * 255.0).astype(np.uint8)
    in_maps = []
    for c in range(NCORES):
        wc = wq_all[c * PPC : (c + 1) * PPC]  # [6, 49, 256, 256] u8
        # -> [row, tap, pair, col, slot]
        wc = wc.reshape(NPAIR, 2, R * R, H, W).transpose(3, 2, 0, 4, 1)
        lc = lpad[c * PPC : (c + 1) * PPC]  # [6, 262, 262]
        lc = lc.reshape(NPAIR, 2, HP, WP).transpose(2, 0, 3, 1)
        in_maps.append(
            {
                "wq": np.ascontiguousarray(wc),
                "lp": np.ascontiguousarray(lc),
            }
        )
    return in_maps


def _unpack_out(full):
    """[NCORES*H, pair, col, slot] f16 -> [B, C, H, W] f32."""
    full = np.asarray(full, np.float32).reshape(NCORES, H, NPAIR, W, 2)
    full = full.transpose(0, 2, 4, 1, 3)  # [core, pair, slot, row, col]
    return full.reshape(B, C, H, W)


def _get_runner():
    """Build the Bass program and ONE sharded jit executable, cached for the
    process.  Repeated kernel() calls reuse the same loaded executable —
    creating a fresh jit per call (as run_bass_kernel_spmd does) loads a new
    executable each time and can wedge the device on the second call."""
    if "runner" in _cache:
        return _cache["runner"]

    import jax
    import concourse.mybir as mybir
    from concourse import bass2jax
    from jax.experimental.shard_map import shard_map
    from jax.sharding import Mesh, NamedSharding, PartitionSpec

    bass2jax.install_neuronx_cc_hook()
    nc = build_nc(reps=1)

    partition_name = nc.partition_id_tensor.name if nc.partition_id_tensor else None
    in_names, out_names, out_avals, zero_outs = [], [], [], []
    for alloc in nc.m.functions[0].allocations:
        if not isinstance(alloc, mybir.MemoryLocationSet):
            continue
        name = alloc.memorylocations[0].name
        if alloc.kind == "ExternalInput":
            if name != partition_name:
                in_names.append(name)
        elif alloc.kind == "ExternalOutput":
            out_names.append(name)
            shape = tuple(alloc.tensor_shape)
            dtype = mybir.dt.np(alloc.dtype)
            out_avals.append(jax.core.ShapedArray(shape, dtype))
            zero_outs.append(np.zeros(shape, dtype))
    n_params = len(in_names)
    all_in_names = list(in_names) + out_names
    if partition_name is not None:
        all_in_names.append(partition_name)

    def _body(*args):
        operands = list(args)
        if partition_name is not None:
            operands.append(bass2jax.partition_id_tensor())
        return tuple(
            bass2jax._bass_exec_p.bind(
                *operands,
                out_avals=tuple(out_avals),
                in_names=tuple(all_in_names),
                out_names=tuple(out_names),
                lowering_input_output_aliases=(),
                sim_require_finite=True,
                sim_require_nnan=True,
                nc=nc,
            )
        )

    devices = jax.devices()[:NCORES]
    mesh = Mesh(np.asarray(devices), ("core",))
    in_specs = (PartitionSpec("core"),) * (n_params + len(out_names))
    out_specs = (PartitionSpec("core"),) * len(out_names)
    sharded = jax.jit(
        shard_map(
            _body, mesh=mesh, in_specs=in_specs, out_specs=out_specs, check_rep=False
        ),
        keep_unused=True,
    )
    sh = NamedSharding(mesh, PartitionSpec("core"))
    zeros_dev = [
        jax.device_put(np.zeros((NCORES * z.shape[0], *z.shape[1:]), z.dtype), sh)
        for z in zero_outs
    ]

    def run(in_maps):
        ins_dev = [
            jax.device_put(
                np.concatenate([in_maps[c][n] for c in range(NCORES)], axis=0), sh
            )
            for n in in_names
        ]
        outs = sharded(*ins_dev, *zeros_dev)
        jax.block_until_ready(outs)
        # one output tensor: per-core [H, pair, col, slot] concatenated
        return np.asarray(outs[0])

    _cache["runner"] = run
    return run


def kernel(latent, weights, window_size):
    r = int(window_size)
    assert r == R, f"kernel hardcoded for window_size={R}, got {r}"

    run = _get_runner()
    in_maps = _prep_inputs(latent, weights)
    full = run(in_maps)
    return _unpack_out(full).astype(np.float32, copy=False)
